# revision 14
# baseline (speedup 1.0000x reference)
"""Trainium2 Bass kernel for the LTC (liquid time-constant) memory cell.

Model (see reference): v-state recurrence over T=128 timesteps, each with 6
ODE unfold iterations:
    v' = (cm_t*v + gl*vl + num_syn) / (cm_t + gl + den_syn + eps)
with 2 recurrent synapses per neuron (self: u, pair: (u+dim)%U) and one
sensory synapse (source d = u%dim).

Sharding: 8 cores; core c owns the 128 neuron *pairs* {u=c*128+p,
u+1024} for p in [0,128), with the FULL batch B=32. Each partition p holds
one pair, so every per-neuron parameter is a per-partition scalar [128,1].
Both halves of a pair live on the same core, so the pair-synapse source is
a local tile — no cross-core traffic in the time loop.

Engine split per unfold (both halves):
 - ACT:    4 sigmoids [128,32] (the loop-carried cycle) + one catted
           *plain* sensory sigmoid [128,64] per timestep reading PSUM
 - DVE:    d1/den STT x2, reciprocal_approx_fast x2, final mul x2,
           sensory ds/nd tensor_scalar (amortized)
 - PE:     q = cm_t*v + nd as diag-matmul PSUM accumulation (off the
           critical cycle), sensory sigmoid args per timestep

The diagonal weight matrices for the PE (diag(cm_t) etc.) are assembled on
the host and DMAed once. The input affine (input_w/input_b) and sensory-mu
fold into scales/biases on the host; the output affine is applied on the
host after gathering.
"""

import numpy as np

import concourse.bacc as bacc
import concourse.mybir as mybir
from concourse import tile
from concourse.tile_rust import add_dep_helper
from concourse.bass_utils import run_bass_kernel_spmd

ODE_UNFOLDS = 6
EPS = 1e-8
B = 32
T = 128
DIM = 1024
U = 2 * DIM
NCORES = 8
P = 128  # partitions = pairs per core

F32 = mybir.dt.float32
AF = mybir.ActivationFunctionType
OP = mybir.AluOpType

# pp column indices (per half; half B adds NPARAM)
# State is carried as w = v + 1 so that w' = (num+den)/den; biases,
# GG and the num-weights are pre-adjusted for the shift.
(C_SIG0, C_B0P, C_SIG1, C_B1P, C_W0, C_W1, C_W0E, C_W1E,
 C_CMT, C_GLV, C_GCME, C_SSIG, C_NSMS, C_SPSW, C_WES,
 C_WPS, C_GGP) = range(17)
NPARAM = 17

# diag matrix slots in the dg input: [cmtA, cmtB, I, ssigA, ssigB,
# nsmsA, nsmsB]
(D_CMT0, D_CMT1, D_EYE, D_SSIG0, D_SSIG1, D_NSMS0, D_NSMS1) = range(7)
NDIAG = 7


def _softplus(x):
    x = x.astype(np.float64)
    return np.log1p(np.exp(-np.abs(x))) + np.maximum(x, 0.0)


def _build_nc(fused_erev=True, wbufs=4, use_recipfast=True, use_pe_q=True,
              use_pe_sens=True):
    nc = bacc.Bacc(trn_type="TRN2")
    xin_d = nc.dram_tensor("xin", [P, T * B], F32, kind="ExternalInput")
    pp_d = nc.dram_tensor("pp", [P, 2 * NPARAM], F32, kind="ExternalInput")
    dg_d = nc.dram_tensor("dg", [P, NDIAG * P], F32, kind="ExternalInput")
    out_d = nc.dram_tensor("out", [P, B], F32, kind="ExternalOutput")

    with tile.TileContext(nc) as tc:
        with tc.tile_pool(name="const", bufs=1) as cpool, \
             tc.tile_pool(name="work", bufs=wbufs) as wpool, \
             tc.tile_pool(name="qp", bufs=2, space="PSUM") as qpool, \
             tc.tile_pool(name="sp", bufs=2, space="PSUM") as spool:
            xin = cpool.tile([P, T * B], F32, tag="xin", name="xin_t")
            pp = cpool.tile([P, 2 * NPARAM], F32, tag="pp", name="pp_t")
            dg = cpool.tile([P, NDIAG * P], F32, tag="dg", name="dg_t")
            nc.sync.dma_start(xin[:], xin_d[:])
            nc.sync.dma_start(pp[:], pp_d[:])
            nc.sync.dma_start(dg[:], dg_d[:])

            def par(h, c):  # per-partition scalar AP for half h param c
                j = h * NPARAM + c
                return pp[:, j:j + 1]

            def diag(i):
                return dg[:, i * P:(i + 1) * P]

            # state tiles (w = v + 1), per half, ping-pong
            v = [[cpool.tile([P, B], F32, tag=f"v{h}{i}",
                             name=f"v{h}{i}") for i in range(2)]
                 for h in range(2)]
            for h in range(2):
                nc.vector.memset(v[h][0][:], 1.0)

            ones = cpool.tile([P, B], F32, tag="ones", name="ones")
            nc.vector.memset(ones[:], 1.0)

            def wtile(tag, cols=B):
                return wpool.tile([P, cols], F32, tag=tag, name=tag)

            def sens_sig(t):
                """Catted sensory sigmoid for both halves; args via PE
                diag matmuls into PSUM, one plain ACT for the sigmoid."""
                xt = xin[:, t * B:(t + 1) * B]
                if not use_pe_sens:
                    sg = wtile("sgc", 2 * B)
                    for h in range(2):
                        nc.scalar.activation(
                            sg[:, h * B:(h + 1) * B], xt, AF.Sigmoid,
                            bias=par(h, C_NSMS), scale=par(h, C_SSIG))
                    return sg
                aps = spool.tile([P, 2 * B], F32, tag="sargp", name="sargp")
                for h in range(2):
                    o = aps[:, h * B:(h + 1) * B]
                    nc.tensor.matmul(o, diag(D_SSIG0 + h), xt,
                                     start=True, stop=False)
                    nc.tensor.matmul(o, diag(D_NSMS0 + h), ones[:],
                                     start=False, stop=True)
                sg = wtile("sgc", 2 * B)
                nc.scalar.activation(sg[:], aps[:], AF.Sigmoid)
                return sg

            def sens_ds(sg, h):
                d_t = wtile(f"ds{h}")
                nc.vector.tensor_scalar(
                    d_t[:], sg[:, h * B:(h + 1) * B], par(h, C_SPSW),
                    par(h, C_GCME), OP.mult, OP.add)
                return d_t

            def sens_nd(sg, h):
                n_t = wtile(f"nd{h}")
                nc.vector.tensor_scalar(
                    n_t[:], sg[:, h * B:(h + 1) * B], par(h, C_WPS),
                    par(h, C_GGP), OP.mult, OP.add)
                return n_t

            def sig(h, slot, vin, scol, bcol):
                s = wtile(f"s{slot}{h}")
                bi = nc.scalar.activation(s[:], vin[:], AF.Sigmoid,
                                          bias=par(h, bcol),
                                          scale=par(h, scol))
                return s, bi

            cur = 0
            sg_c = sens_sig(0)
            ds = [sens_ds(sg_c, h) for h in range(2)]
            nd = [sens_nd(sg_c, h) for h in range(2)]
            s0A, _ = sig(0, 0, v[0][0], C_SIG0, C_B0P)
            s1A, _ = sig(0, 1, v[1][0], C_SIG1, C_B1P)
            s0B, _ = sig(1, 0, v[1][0], C_SIG0, C_B0P)
            s1B, _ = sig(1, 1, v[0][0], C_SIG1, C_B1P)
            sg_n = None
            ds_n = [None, None]
            nd_n = [None, None]
            for t in range(T):
                more = t + 1 < T
                for k in range(ODE_UNFOLDS):
                    # ---- q = cmt*v + nd (off the critical cycle) ----
                    if use_pe_q:
                        qps = qpool.tile([P, 2 * B], F32, tag="qps",
                                         name="qps")
                        for h in range(2):
                            o = qps[:, h * B:(h + 1) * B]
                            nc.tensor.matmul(o, diag(D_CMT0 + h),
                                             v[h][cur][:],
                                             start=True, stop=False)
                            nc.tensor.matmul(o, diag(D_EYE), nd[h][:],
                                             start=False, stop=True)
                        qA = qps[:, 0:B]
                        qB = qps[:, B:2 * B]
                    else:
                        qAt = wtile("qA")
                        qBt = wtile("qB")
                        nc.vector.scalar_tensor_tensor(
                            qAt[:], v[0][cur][:], par(0, C_CMT),
                            nd[0][:], OP.mult, OP.add)
                        nc.vector.scalar_tensor_tensor(
                            qBt[:], v[1][cur][:], par(1, C_CMT),
                            nd[1][:], OP.mult, OP.add)
                        qA, qB = qAt[:], qBt[:]
                    # ---- den chain on DVE ----
                    d1A = wtile("d1A")
                    d1B = wtile("d1B")
                    nc.vector.scalar_tensor_tensor(
                        d1A[:], s0A[:], par(0, C_W0),
                        ds[0][:], OP.mult, OP.add)
                    nc.vector.scalar_tensor_tensor(
                        d1B[:], s1B[:], par(1, C_W1),
                        ds[1][:], OP.mult, OP.add)
                    if not fused_erev:
                        m1A = wtile("m1A")
                        m1B = wtile("m1B")
                        nc.vector.scalar_tensor_tensor(
                            m1A[:], s0A[:], par(0, C_W0E),
                            qA, OP.mult, OP.add)
                        nc.vector.scalar_tensor_tensor(
                            m1B[:], s1B[:], par(1, C_W1E),
                            qB, OP.mult, OP.add)
                    # sensory fillers for next timestep (DVE slack)
                    if more and k == 3:
                        for h in range(2):
                            ds_n[h] = sens_ds(sg_n, h)
                            nd_n[h] = sens_nd(sg_n, h)
                    denA = wtile("denA")
                    denB = wtile("denB")
                    rA = wtile("rA")
                    rB = wtile("rB")
                    nc.vector.scalar_tensor_tensor(
                        denA[:], s1A[:], par(0, C_W1),
                        d1A[:], OP.mult, OP.add)
                    nc.vector.scalar_tensor_tensor(
                        denB[:], s0B[:], par(1, C_W0),
                        d1B[:], OP.mult, OP.add)
                    if use_recipfast:
                        nc.vector.reciprocal_approx_fast(rA[:], denA[:])
                        nc.vector.reciprocal_approx_fast(rB[:], denB[:])
                    else:
                        nc.vector.reciprocal(rA[:], denA[:])
                        nc.vector.reciprocal(rB[:], denB[:])
                    if fused_erev:
                        mA, mB = qA, qB
                    else:
                        mAt = wtile("mA")
                        mBt = wtile("mB")
                        nc.vector.scalar_tensor_tensor(
                            mAt[:], s1A[:], par(0, C_W1E),
                            m1A[:], OP.mult, OP.add)
                        nc.vector.scalar_tensor_tensor(
                            mBt[:], s0B[:], par(1, C_W0E),
                            m1B[:], OP.mult, OP.add)
                        mA, mB = mAt[:], mBt[:]
                    nxt = 1 - cur
                    nc.vector.tensor_mul(v[0][nxt][:], mA, rA[:])
                    n_s0A, _ = sig(0, 0, v[0][nxt], C_SIG0, C_B0P)
                    nc.vector.tensor_mul(v[1][nxt][:], mB, rB[:])
                    n_s1A, _ = sig(0, 1, v[1][nxt], C_SIG1, C_B1P)
                    n_s0B, bi_s0B = sig(1, 0, v[1][nxt], C_SIG0, C_B0P)
                    n_s1B, bi_s1B = sig(1, 1, v[0][nxt], C_SIG1, C_B1P)
                    add_dep_helper(bi_s0B.ins, bi_s1B.ins, sync=True,
                                   reason="s1B off the critical ACT slot")
                    s0A, s1A = n_s0A, n_s1A
                    s0B, s1B = n_s0B, n_s1B
                    cur = nxt
                    # mid-timestep sensory sigmoid (ACT slack)
                    if more and k == 2:
                        sg_n = sens_sig(t + 1)
                if more:
                    for h in range(2):
                        ds[h] = ds_n[h]
                        nd[h] = nd_n[h]

            nc.sync.dma_start(out_d[:], v[0][cur][:])
    nc.compile()
    return nc


_NC_CACHE = {}


def _flags():
    import os
    return dict(
        use_recipfast=os.environ.get("K_RECIPFAST", "1") == "1",
        use_pe_q=os.environ.get("K_PE_Q", "1") == "1",
        use_pe_sens=os.environ.get("K_PE_SENS", "1") == "1",
    )


def _get_nc(fused_erev=True):
    fl = _flags()
    key = (fused_erev, tuple(sorted(fl.items())))
    if key not in _NC_CACHE:
        _NC_CACHE[key] = _build_nc(fused_erev, **fl)
    return _NC_CACHE[key]


def _host_params(c, gleak, vleak, cm, w, sigma, mu, erev,
                 sens_w, sens_sigma, sens_mu, sens_erev,
                 input_w, input_b):
    """pp [128, 2*NPARAM] and dg [128, NDIAG*128] for core c."""
    d = c * P + np.arange(P)
    pp = np.zeros((P, 2 * NPARAM), np.float32)
    for h in range(2):
        u = h * DIM + d
        sp_w = _softplus(w[u])                       # [P,2]
        sp_gl = _softplus(gleak[u])
        cmt = _softplus(cm[u]) * ODE_UNFOLDS
        o = h * NPARAM
        # state shift w = v + 1: sigmoid biases absorb -sigma, GG absorbs
        # -cmt (so q = cmt*w + ND == cmt*v + NS + DS).
        pp[:, o + C_SIG0] = sigma[u, 0]
        pp[:, o + C_B0P] = -(mu[u, 0] + 1.0) * sigma[u, 0]
        pp[:, o + C_SIG1] = sigma[u, 1]
        pp[:, o + C_B1P] = -(mu[u, 1] + 1.0) * sigma[u, 1]
        pp[:, o + C_W0] = sp_w[:, 0]
        pp[:, o + C_W1] = sp_w[:, 1]
        pp[:, o + C_W0E] = sp_w[:, 0] * (1.0 + erev[u, 0])
        pp[:, o + C_W1E] = sp_w[:, 1] * (1.0 + erev[u, 1])
        pp[:, o + C_CMT] = cmt
        pp[:, o + C_GLV] = sp_gl * vleak[u]
        pp[:, o + C_GCME] = cmt + sp_gl + EPS
        pp[:, o + C_SSIG] = sens_sigma[u] * input_w[d]
        pp[:, o + C_NSMS] = (input_b[d] - sens_mu[u]) * sens_sigma[u]
        pp[:, o + C_SPSW] = _softplus(sens_w[u])
        pp[:, o + C_WES] = _softplus(sens_w[u]) * sens_erev[u]
        pp[:, o + C_WPS] = pp[:, o + C_SPSW] + pp[:, o + C_WES]
        pp[:, o + C_GGP] = pp[:, o + C_GCME] + pp[:, o + C_GLV] - cmt
    dgm = np.zeros((P, NDIAG, P), np.float32)
    rng = np.arange(P)
    dgm[rng, D_CMT0, rng] = pp[:, C_CMT]
    dgm[rng, D_CMT1, rng] = pp[:, NPARAM + C_CMT]
    dgm[rng, D_EYE, rng] = 1.0
    dgm[rng, D_SSIG0, rng] = pp[:, C_SSIG]
    dgm[rng, D_SSIG1, rng] = pp[:, NPARAM + C_SSIG]
    dgm[rng, D_NSMS0, rng] = pp[:, C_NSMS]
    dgm[rng, D_NSMS1, rng] = pp[:, NPARAM + C_NSMS]
    return pp, dgm.reshape(P, NDIAG * P)


def kernel(inputs, gleak, vleak, cm, w, sigma, mu, erev,
           sens_w, sens_sigma, sens_mu, sens_erev,
           input_w, input_b, output_w, output_b, _trace=False):
    inputs = np.asarray(inputs, np.float32)
    args = dict(gleak=np.asarray(gleak, np.float32),
                vleak=np.asarray(vleak, np.float32),
                cm=np.asarray(cm, np.float32),
                w=np.asarray(w, np.float32),
                sigma=np.asarray(sigma, np.float32),
                mu=np.asarray(mu, np.float32),
                erev=np.asarray(erev, np.float32),
                sens_w=np.asarray(sens_w, np.float32),
                sens_sigma=np.asarray(sens_sigma, np.float32),
                sens_mu=np.asarray(sens_mu, np.float32),
                sens_erev=np.asarray(sens_erev, np.float32),
                input_w=np.asarray(input_w, np.float32),
                input_b=np.asarray(input_b, np.float32))

    in_maps = []
    for c in range(NCORES):
        xc = inputs[:, :, c * P:(c + 1) * P]          # [B,T,P]
        xin = np.ascontiguousarray(
            xc.transpose(2, 1, 0).reshape(P, T * B))  # [P, t*B+b]
        pp, dgm = _host_params(c, **args)
        in_maps.append({"xin": xin, "pp": pp, "dg": dgm})

    fused = bool(np.allclose(args["erev"], -1.0))
    nc = _get_nc(fused)
    res = run_bass_kernel_spmd(nc, in_maps, core_ids=list(range(NCORES)),
                               trace=_trace)

    out = np.zeros((B, DIM), np.float32)
    for c in range(NCORES):
        out[:, c * P:(c + 1) * P] = res.results[c]["out"].T
    out = out - 1.0  # state was carried as w = v + 1
    out = out * np.asarray(output_w, np.float32) + np.asarray(output_b, np.float32)
    if _trace:
        kernel.last_results = res
    return out
